# revision 1
# baseline (speedup 1.0000x reference)
"""Trainium2 Bass kernel for nn_Attention (B=8, L=2048, D=64).

Reference (per batch b):
    BZ = x @ B_w.T + B_b
    CZ = x @ C_w.T + C_b
    scores = BZ @ CZ.T              # (L, L)
    attn = relu(scores)
    attn = attn / (attn.sum(axis=-2, keepdims=True) + EPS)   # column-sum norm
    VZ = x @ V_w.T + V_b
    out = x + attn @ VZ

Strategy (one batch per NeuronCore, 8 cores, no cross-core comms):
  * Biases folded via an augmented feature dim: x_aug = [x, 1] (65) and
    host-stacked weights [W.T; b] (65 x 64).
  * x transposed on the PE (16 [128,64] transposes) into x_augT [65, L].
  * Projections BZ^T / CZ^T computed TWICE (output partitions 0-63 and
    64-127 via PE column tiling) so the K=64 scores matmuls can be
    row-packed: two m-chunks run concurrently in PE row groups 0-1/2-3.
  * S^T orientation puts the column-normalization axis on PSUM
    partitions; relu + column-sums fuse into the PSUM->SBUF evacuation
    (ACT activation accum_out / DVE tensor_scalar accum_out), balanced
    across both engines.
  * Normalization folds into VZ rows: O^T = (VZ*recip)^T @ A^T,
    accumulated in PSUM over m-chunks, column-packed into [128, 1024]
    (l lower half on partitions 0-63, upper on 64-127).
  * Software-pipelined emission: O matmuls trail the scores of the next
    chunk pair so the in-order PE never stalls on the relu/normalize
    chain; a dummy-matmul warmup burst overlaps the input DMA to lift
    the PE clock gate (HAM) to 2.4 GHz before real work starts.
  * All PE matmuls run in fp16 (fp32 PSUM accumulation): same ~10-bit
    mantissa class as fp32r but full rate, cheap weight loads, and
    row/col tile-packing work. Measured rel err ~1.4e-4 of output scale.
"""

import os
import sys

sys.path.insert(0, "/opt/trn_rl_repo")

import numpy as np

import concourse.bacc as bacc
import concourse.tile as tile
from concourse import mybir
from concourse import bass_utils

L = 2048
D = 64
DA = D + 1          # augmented feature dim
P = 128
NCH = L // P        # 16 m-chunks
SL = 512            # matmul moving-slice width
NSL = L // SL       # 4 slices
EU = 1024           # relu-evacuation unit width (2 slices)
EPS = 1e-8
N_CORES = 8

F32 = mybir.dt.float32
F32R = mybir.dt.float32r
F16 = mybir.dt.float16


def _attention_kernel(ctx, tc, y_ap, x_ap, b_ap, c_ap, v_ap, ones_ap, at_dt):
    nc = tc.nc
    Relu = mybir.ActivationFunctionType.Relu
    Copy = mybir.ActivationFunctionType.Copy
    AX = mybir.AxisListType.X
    Alu = mybir.AluOpType

    consts = ctx.enter_context(tc.tile_pool(name="consts", bufs=1))
    bigs = ctx.enter_context(tc.tile_pool(name="bigs", bufs=1))
    at_pool = ctx.enter_context(tc.tile_pool(name="at", bufs=5))
    small = ctx.enter_context(tc.tile_pool(name="small", bufs=6))

    # Weights: DMA straight into f32r tiles (PE rounds internally)
    b_sb = consts.tile([DA, D], F16)
    nc.sync.dma_start(out=b_sb, in_=b_ap)
    c_sb = consts.tile([DA, D], F16)
    nc.sync.dma_start(out=c_sb, in_=c_ap)
    v_sb = consts.tile([DA, D], F16)
    nc.sync.dma_start(out=v_sb, in_=v_ap)
    ident = consts.tile([P, P], F32)
    from concourse.masks import make_identity
    make_identity(nc, ident)

    # x natural layout [p, chunk, d], l = chunk*128 + p
    x_nat = bigs.tile([P, NCH, D], F32)
    x_r = x_ap.rearrange("(c p) d -> p c d", p=P)
    for g in range(4):
        nc.sync.dma_start(out=x_nat[:, 4 * g : 4 * (g + 1), :],
                          in_=x_r[:, 4 * g : 4 * (g + 1), :])

    xT = bigs.tile([DA, L], F16)          # x_aug^T
    nc.sync.dma_start(out=xT[D:DA, :], in_=ones_ap)
    bz = bigs.tile([P, L], F16)           # BZ^T duplicated on both halves
    cz = bigs.tile([P, L], F16)           # CZ^T duplicated on both halves
    vz_sb = bigs.tile([P, NCH, D], F32)    # VZ natural
    out_sb = bigs.tile([P, NCH, D], F32)

    # ---------------- PE warmup (HAM un-throttle) ----------------
    # ~5us of dense dummy matmuls overlapping the input DMA: lifts the PE
    # clock gate from 1.2 to 2.4 GHz before real work starts.
    wu_a = consts.tile([P, SL], F16)
    nc.vector.memset(wu_a, 0.25)
    wu_res = consts.tile([P, 1], F32)
    with tc.tile_pool(name="pw", bufs=2, space="PSUM") as pw_pool:
        for i in range(20):
            pw = pw_pool.tile([P, SL], F32, tag="pw")
            nc.tensor.matmul(pw, wu_a[:, 0:P], wu_a, start=True, stop=True)
            nc.vector.tensor_copy(wu_res, pw[:, 0:1])

    # ---------------- prologue ----------------
    with tc.tile_pool(name="pt", bufs=2, space="PSUM") as pt_pool, \
         tc.tile_pool(name="pp", bufs=3, space="PSUM") as pp_pool, \
         tc.tile_pool(name="pv", bufs=2, space="PSUM") as pv_pool:
        # x^T via PE transposes, 4 chunks per PSUM bank
        for g in range(NCH // 4):
            pt = pt_pool.tile([D, 4, P], F32)
            for j in range(4):
                c = 4 * g + j
                nc.tensor.transpose(pt[:, j, :], x_nat[:, c, :], ident)
            nc.scalar.activation(
                out=xT[0:D, SL * g : SL * (g + 1)].rearrange(
                    "e (a p) -> e a p", a=4),
                in_=pt, func=Copy)
        # BZ^T / CZ^T on partitions 0-63; duplicate to 64-127 via SBUF DMA
        # (fp32r matmuls cannot target PSUM col groups 2-3)
        for w_sb, dst, eng in ((b_sb, bz, "v"), (c_sb, cz, "s")):
            for j in range(NSL):
                pp = pp_pool.tile([P, SL], F32)
                nc.tensor.matmul(pp[0:D, :], w_sb,
                                 xT[:, SL * j : SL * (j + 1)],
                                 start=True, stop=True)
                if eng == "v":
                    nc.vector.tensor_copy(dst[0:D, SL * j : SL * (j + 1)],
                                          pp[0:D, :])
                else:
                    nc.scalar.activation(
                        out=dst[0:D, SL * j : SL * (j + 1)], in_=pp[0:D, :],
                        func=Copy)
                nc.gpsimd.dma_start(out=dst[D : 2 * D, SL * j : SL * (j + 1)],
                                     in_=dst[0:D, SL * j : SL * (j + 1)])
        # VZ natural: 8 chunks per PSUM bank
        for g in range(2):
            pv = pv_pool.tile([P, 8, D], F32)
            for j in range(8):
                c = 8 * g + j
                nc.tensor.matmul(pv[:, j, :],
                                 xT[:, P * c : P * (c + 1)],
                                 v_sb, start=True, stop=True)
            nc.scalar.activation(out=vz_sb[:, 8 * g : 8 * (g + 1), :],
                                 in_=pv, func=Copy)

    # ---------------- main loop ----------------
    # Chunk PAIR (cA rows 0-63, cB rows 64-127, row-packed so both score
    # matmuls run concurrently in separate PE row groups), relu+colsum
    # evacuation, normalization folded into VZ; O^T accumulation emitted
    # one pair behind so the in-order PE never waits on the chain.
    def emit_chain(cA, cB, cs2A, cs2B):
        # cs = sum(cs2)+EPS fused in one DVE op per chunk, shared recip
        csAB = small.tile([P, 2], F32, tag="csAB")
        sc = small.tile([P, 4], F32, tag="scratch")
        nc.vector.tensor_scalar(out=sc[:, 0:2], in0=cs2A,
                                scalar1=1.0, scalar2=EPS,
                                op0=Alu.mult, op1=Alu.add,
                                accum_out=csAB[:, 0:1])
        nc.vector.tensor_scalar(out=sc[:, 2:4], in0=cs2B,
                                scalar1=1.0, scalar2=EPS,
                                op0=Alu.mult, op1=Alu.add,
                                accum_out=csAB[:, 1:2])
        recip = small.tile([P, 2], F32, tag="recip")
        nc.vector.reciprocal(recip, csAB)
        vzsA = small.tile([P, D], at_dt, tag="vzsA")
        nc.scalar.activation(out=vzsA, in_=vz_sb[:, cA, :], func=Copy,
                             scale=recip[:, 0:1])
        vzsB = small.tile([P, D], at_dt, tag="vzsB")
        nc.scalar.activation(out=vzsB, in_=vz_sb[:, cB, :], func=Copy,
                             scale=recip[:, 1:2])
        return vzsA, vzsB

    def emit_o(c, at, vzs):
        # column-packed: j 0/1 -> partitions 0-63, j 2/3 -> 64-127;
        # interleave col groups so both halves overlap on the PE
        for j in (0, 2, 1, 3):
            if j < 2:
                out_ap = po[0:D, SL * j : SL * (j + 1)]
            else:
                out_ap = po[D : 2 * D, SL * (j - 2) : SL * (j - 1)]
            nc.tensor.matmul(out_ap, vzs, at[:, SL * j : SL * (j + 1)],
                             start=(c == 0), stop=(c == NCH - 1))

    with tc.tile_pool(name="po", bufs=1, space="PSUM") as po_pool:
        po = po_pool.tile([P, EU], F32)    # O^T column-packed (2 banks)
        with tc.tile_pool(name="ps", bufs=3, space="PSUM") as ps_pool:
            prev = None
            for p in range(NCH // 2):
                cA, cB = 2 * p, 2 * p + 1
                atA = at_pool.tile([P, L], at_dt, tag="at")
                atB = at_pool.tile([P, L], at_dt, tag="at")
                cs2A = small.tile([P, 2], F32, tag="cs2A")
                cs2B = small.tile([P, 2], F32, tag="cs2B")
                pstiles = {}
                for u in range(2):
                    psA = ps_pool.tile([P, EU], F32, tag="ps")
                    psB = ps_pool.tile([P, EU], F32, tag="ps")
                    pstiles[("A", u)] = psA
                    pstiles[("B", u)] = psB
                    for jj in range(2):
                        j = 2 * u + jj
                        nc.tensor.matmul(psA[:, SL * jj : SL * (jj + 1)],
                                         cz[0:D, P * cA : P * (cA + 1)],
                                         bz[0:D, SL * j : SL * (j + 1)],
                                         start=True, stop=True)
                        nc.tensor.matmul(psB[:, SL * jj : SL * (jj + 1)],
                                         cz[D : 2 * D, P * cB : P * (cB + 1)],
                                         bz[D : 2 * D, SL * j : SL * (j + 1)],
                                         start=True, stop=True)
                # relu + column-sum evacuation: ACT unit u=0, DVE unit
                # u=1 for each chunk
                for nm, at, cs2 in (("A", atA, cs2A), ("B", atB, cs2B)):
                    nc.scalar.activation(
                        out=at[:, 0:EU], in_=pstiles[(nm, 0)],
                        func=Relu, accum_out=cs2[:, 0:1])
                    nc.vector.tensor_scalar(
                        out=at[:, EU : 2 * EU], in0=pstiles[(nm, 1)],
                        scalar1=0.0, scalar2=None,
                        op0=Alu.max, op1=Alu.add,
                        accum_out=cs2[:, 1:2])
                vzsA, vzsB = emit_chain(cA, cB, cs2A, cs2B)
                if prev is not None:
                    emit_o(prev[0], prev[1], prev[2])
                    emit_o(prev[3], prev[4], prev[5])
                prev = (cA, atA, vzsA, cB, atB, vzsB)
            emit_o(prev[0], prev[1], prev[2])
            emit_o(prev[3], prev[4], prev[5])

        # ---------------- epilogue ----------------
        with tc.tile_pool(name="pf", bufs=2, space="PSUM") as pf_pool:
            ot = bigs.tile([P, EU], F32)
            nc.scalar.activation(out=ot[:, 0 : EU // 2],
                                 in_=po[:, 0 : EU // 2], func=Copy)
            nc.vector.tensor_copy(ot[:, EU // 2 : EU], po[:, EU // 2 : EU])
            for g in range(NCH // 4):
                pf = pf_pool.tile([P, 4, D], F32)
                for j in range(4):
                    c = 4 * g + j
                    half, col = divmod(c, 8)
                    r0 = D * half
                    nc.tensor.transpose(
                        pf[:, j, :], ot[r0 : r0 + D, P * col : P * (col + 1)],
                        ident[r0 : r0 + D, r0 : r0 + D])
                nc.vector.tensor_add(out_sb[:, 4 * g : 4 * (g + 1), :],
                                     x_nat[:, 4 * g : 4 * (g + 1), :], pf)
                nc.gpsimd.dma_start(
                    out=y_ap.rearrange("(c p) d -> p c d", p=P)[
                        :, 4 * g : 4 * (g + 1), :],
                    in_=out_sb[:, 4 * g : 4 * (g + 1), :])


_CACHE = {}


def _build(at_dt_name="f16"):
    key = ("nc", at_dt_name)
    if key in _CACHE:
        return _CACHE[key]
    at_dt = {"f32r": F32R, "bf16": mybir.dt.bfloat16, "f16": F16}[at_dt_name]
    nc = bacc.Bacc("TRN2", target_bir_lowering=False, debug=False,
                   enable_asserts=False, num_devices=1)
    x = nc.dram_tensor("x", (L, D), F32, kind="ExternalInput").ap()
    b = nc.dram_tensor("b_augt", (DA, D), F16, kind="ExternalInput").ap()
    c = nc.dram_tensor("c_augt", (DA, D), F16, kind="ExternalInput").ap()
    v = nc.dram_tensor("v_augt", (DA, D), F16, kind="ExternalInput").ap()
    ones = nc.dram_tensor("ones", (1, L), F16, kind="ExternalInput").ap()
    y = nc.dram_tensor("y", (L, D), F32, kind="ExternalOutput").ap()
    from contextlib import ExitStack
    with tile.TileContext(nc) as tc, ExitStack() as ctx:
        _attention_kernel(ctx, tc, y, x, b, c, v, ones, at_dt)
    nc.compile()
    _CACHE[key] = nc
    return nc


def _fold_weights(B_w, B_b, C_w, C_b, V_w, V_b):
    def aug(w, bias):
        full = np.concatenate(
            [np.asarray(w, np.float32).T, np.asarray(bias, np.float32)[None, :]],
            axis=0)
        return full.astype(np.float16)
    return aug(B_w, B_b), aug(C_w, C_b), aug(V_w, V_b)


def run(inputs, trace=False, tmpdir=None, at_dt="f16"):
    nc = _build(at_dt)
    x = np.ascontiguousarray(np.asarray(inputs["x"], dtype=np.float32))
    b_augt, c_augt, v_augt = _fold_weights(
        inputs["B_w"], inputs["B_b"], inputs["C_w"], inputs["C_b"],
        inputs["V_w"], inputs["V_b"])
    ones = np.ones((1, L), np.float16)
    in_maps = [{"x": x[i], "b_augt": b_augt, "c_augt": c_augt,
                "v_augt": v_augt, "ones": ones} for i in range(N_CORES)]
    res = bass_utils.run_bass_kernel_spmd(nc, in_maps,
                                          core_ids=list(range(N_CORES)),
                                          trace=trace, tmpdir=tmpdir)
    out = np.stack([res.results[i]["y"] for i in range(N_CORES)], axis=0)
    return out, res


def kernel(**inputs) -> np.ndarray:
    out, _ = run(inputs, trace=False)
    return out

